# revision 22
# baseline (speedup 1.0000x reference)
# Trainium2 Bass kernel for nn_AlignmentEncoder (RAD-TTS style alignment encoder).
#
# Math (per sample):
#   k_spk = kspk_w @ spk + kspk_b ; q_spk = qspk_w @ spk + qspk_b
#   keys_enc = Conv1x(ReLU(Conv3(keys + k_spk)))                      [80, 512]
#   queries_enc = Conv1x(ReLU(Conv1x(ReLU(Conv3(queries + q_spk)))))  [80, 2048]
#   logits = -T*(q2 + k2 - 2 qk) ; lp = log_softmax(logits) + log(prior + 1e-8)
#   attn = softmax(where(mask, -1e9, lp))
#
# Device-side simplifications (exact up to fp rounding):
#   * q2 (per-row constant) cancels in both log_softmax and softmax -> dropped.
#   * L := 2T*qk - T*k2 computed as ONE matmul with K=97 (k2 folded as an
#     extra contraction row at partition 96 against a ones-row in the query
#     operand; partitions 80..95 are zeroed).
#   * no row-max subtraction: L ranges ~[-1, +1], exp() is safe.
#   * exp(L + log p) == exp(L) * p   ->  no log-prior add per tile:
#       el = exp(L), s0 = sum(el);  ts = el * p;  lp_out = ln(ts / s0)
#       tm = ts * mask01;  attn = tm / sum(tm)   (the 1/s0 factor cancels)
#   * conv k=3 edges via ragged per-tap matmul output ranges (PSUM
#     has_written handles partial-range accumulation) - no zero padding.
#   * Exp and Ln batched into separate passes per sample (ACT function
#     tables for exp and ln live in different table sets; interleaving
#     would reload the table per instruction, ~1.3us each).
#
# Sharding: pure data-parallel, batch 32 = 8 cores x 4 samples. No collectives.
import sys

if "/opt/trn_rl_repo" not in sys.path:
    sys.path.insert(0, "/opt/trn_rl_repo")

import numpy as np
import ml_dtypes

import concourse.bass as bass
import concourse.bacc as bacc
import concourse.tile as tile
from concourse import mybir
from concourse.bass_utils import run_bass_kernel_spmd

BF = mybir.dt.bfloat16
F32 = mybir.dt.float32
F16 = mybir.dt.float16
F8 = mybir.dt.float8e4
NBF = ml_dtypes.bfloat16
NF8 = ml_dtypes.float8_e4m3
W1K_SCALE = 8.0
XK_SCALE = 4.0

TEMP = 0.0005
B, T1, T2 = 32, 2048, 512
CM, CK = 80, 512           # n_mel/n_att, n_text
NCORES, BL = 8, 4          # cores, samples per core
NT1 = T1 // 128            # 16 t1-tiles per sample
ACT = mybir.ActivationFunctionType
ALU = mybir.AluOpType

_nc_cache = None


def conv3_ranges(t_total):
    """Per-tap (d, out_lo, out_hi, in_lo) for a k=3 'same' conv as matmuls.
    Tap d multiplies x[t + d - 1]; ragged output ranges at the edges."""
    out = []
    for d in (1, 0, 2):  # d=1 first: full range, so start=True covers the bank
        lo = max(0, 1 - d)                 # t+d-1 >= 0
        hi = min(t_total, t_total + 1 - d)  # t+d-1 < t_total
        out.append((d, lo, hi, lo + d - 1))
    return out


def build_nc():
    nc = bacc.Bacc("TRN2", target_bir_lowering=False, debug=False,
                   num_devices=NCORES)

    def din(name, shape, dt):
        return nc.dram_tensor(name, list(shape), dt, kind="ExternalInput").ap()

    def dout(name, shape, dt):
        return nc.dram_tensor(name, list(shape), dt, kind="ExternalOutput").ap()

    # -------- external tensors (layouts are partition-first; see host prep)
    qin = din("qin", (BL, CM, T1), BF)              # queries (bf16)
    kin = din("kin", (BL, 128, 4, T2), BF)          # keys [b, p, ci_blk, t]
    prior = din("prior", (BL, NT1, 128, T2), BF)    # attn_prior + 1e-8, tiled
    msk = din("msk", (BL, T2), BF)                  # 1.0 valid / 0.0 masked
    spkT = din("spkT", (128, 4, BL), BF)            # speaker_embed^T blocks
    w1k = din("w1k", (128, 3, 4, 8, 128), F8)       # kw1 [ci_p, d, ci_blk, co_blk, co]
    pk128 = din("pk128", (128, 3008), BF)           # wkspk | w2k | wqspk packed
    pk80 = din("pk80", (CM, 720), BF)               # w1q | w2q | w3q packed
    pkf = din("pkf", (128, 19), F32)                # biases packed
    out_lp = dout("out_lp", (BL, NT1, 128, T2), F16)
    out_at = dout("out_at", (BL, NT1, 128, T2), F16)

    with tile.TileContext(nc) as tc:
        import contextlib
        with contextlib.ExitStack() as ctx:
            wts = ctx.enter_context(tc.tile_pool(name="wts", bufs=1))
            samp2 = ctx.enter_context(tc.tile_pool(name="samp2", bufs=2))
            samp1 = ctx.enter_context(tc.tile_pool(name="samp1", bufs=1))
            samp2b = ctx.enter_context(tc.tile_pool(name="samp2b", bufs=2))
            outp = ctx.enter_context(tc.tile_pool(name="outp", bufs=2))
            soft = ctx.enter_context(tc.tile_pool(name="soft", bufs=4))
            stats = ctx.enter_context(tc.tile_pool(name="stats", bufs=2))
            ps_l = ctx.enter_context(tc.tile_pool(name="ps_l", bufs=3, space="PSUM"))
            ps_k1 = ctx.enter_context(tc.tile_pool(name="ps_k1", bufs=2, space="PSUM"))
            ps_m = ctx.enter_context(tc.tile_pool(name="ps_m", bufs=1, space="PSUM"))
            ps_q = ctx.enter_context(tc.tile_pool(name="ps_q", bufs=2, space="PSUM"))

            # -------- static weights into SBUF
            def wtile(ap_in, shape, dt, tag):
                t = wts.tile(list(shape), dt, tag=tag)
                nc.sync.dma_start(t[:], ap_in[:])
                return t

            spkT_s = wtile(spkT, (128, 4, BL), BF, "spkT")
            pkf_s = wtile(pkf, (128, 19), F32, "pkf")
            pk128_s = wtile(pk128, (128, 3008), BF, "pk128")
            pk80_s = wtile(pk80, (CM, 720), BF, "pk80")
            w1k_s = wtile(w1k, (128, 3, 4, 8, 128), F8, "w1k")
            wkspk_s = pk128_s[:, 0:2048].rearrange("p (a b c) -> p a b c",
                                                   a=4, b=4)
            w2k_s = pk128_s[:, 2048:2688].rearrange("p (a b) -> p a b", a=8)
            wqspk_s = pk128_s[:, 2688:3008].rearrange("p (a b) -> p a b", a=4)
            w1q_s = pk80_s[:, 0:480].rearrange("p (a b c) -> p a b c", a=3, b=2)
            w2q_s = pk80_s[:, 480:640].rearrange("p (a b) -> p a b", a=2)
            w3q_s = pk80_s[:, 640:720]
            bk1_s = pkf_s[:, 0:8]
            bkspk_s = pkf_s[:, 8:12]
            bk2_s = pkf_s[0:CM, 12:13]
            bk2s_s = pkf_s[0:CM, 13:14]
            bq1_s = pkf_s[0:CM, 14:16]
            bq2_s = pkf_s[0:CM, 16:17]
            bq3_s = pkf_s[0:CM, 17:18]
            bqspk_s = pkf_s[0:CM, 18:19]

            ones80 = wts.tile([CM, 1], BF, tag="ones80")
            nc.gpsimd.memset(ones80[:], 1.0)

            # static double-buffered augmented encoder outputs; filler rows
            # (80..95 zero) + ones row (96) written once.
            qencA2, kencA2 = [], []
            for i in range(2):
                qe = wts.tile([97, T1], BF, tag=f"qencA{i}")
                nc.gpsimd.memset(qe[64:96, :], 0.0)
                nc.gpsimd.memset(qe[96:97, :], 1.0)
                qencA2.append(qe)
                ke = wts.tile([97, T2], BF, tag=f"kencA{i}")
                nc.gpsimd.memset(ke[64:96, :], 0.0)
                kencA2.append(ke)

            # -------- speaker projections (all local samples at once)
            kspk_s = wts.tile([128, 4, BL], F32, tag="kspk")   # [c_p, c_blk, b]
            qspk_s = wts.tile([CM, BL], F32, tag="qspk")
            for cb in range(4):
                ps = ps_k1.tile([128, BL], F32, tag="pk1")
                for jb in range(4):
                    nc.tensor.matmul(ps[:], wkspk_s[:, jb, cb, :], spkT_s[:, jb, :],
                                     start=(jb == 0), stop=(jb == 3))
                nc.scalar.activation(kspk_s[:, cb, :], ps[:], ACT.Identity,
                                     bias=bkspk_s[:, cb:cb + 1])
            psq = ps_m.tile([CM, BL], F32, tag="pk2")
            for jb in range(4):
                nc.tensor.matmul(psq[:], wqspk_s[:, jb, :], spkT_s[:, jb, :],
                                 start=(jb == 0), stop=(jb == 3))
            nc.scalar.activation(qspk_s[:], psq[:], ACT.Identity, bias=bqspk_s[:])

            def encoders(b):
                qencA = qencA2[b % 2]
                kencA = kencA2[b % 2]
                # ================= keys encoder =================
                xk = samp2.tile([128, 4, T2], BF, tag="xk")
                nc.sync.dma_start(xk[:], kin[b])
                xk8 = samp2.tile([128, 4, T2], F8, tag="xk8")
                for cb in range(4):
                    # xk8 = (keys + kspk) * XK_SCALE, in fp8 for DoubleRow
                    nc.vector.tensor_scalar(xk8[:, cb, :], xk[:, cb, :],
                                            kspk_s[:, cb, b:b + 1], XK_SCALE,
                                            op0=ALU.add, op1=ALU.mult)
                h1k = samp1.tile([128, 8, T2], BF, tag="h1k")
                for ob in range(8):
                    pk = ps_k1.tile([128, T2], F32, tag="pk1")
                    first = True
                    for cbp in range(2):
                        for d, lo, hi, ilo in conv3_ranges(T2):
                            nc.tensor.matmul(
                                pk[:, lo:hi],
                                w1k_s[:, d, 2 * cbp:2 * cbp + 2, ob, :],
                                xk8[:, 2 * cbp:2 * cbp + 2, ilo:ilo + (hi - lo)],
                                start=first, stop=(cbp == 1 and d == 2),
                                perf_mode=mybir.MatmulPerfMode.DoubleRow)
                            first = False
                    nc.vector.tensor_scalar(h1k[:, ob, :], pk[:],
                                            bk1_s[:, ob:ob + 1], 0.0,
                                            op0=ALU.add, op1=ALU.max)
                pk2 = ps_m.tile([CM, T2], F32, tag="pk2")
                for cb in range(8):
                    nc.tensor.matmul(pk2[:], w2k_s[:, cb, :], h1k[:, cb, :],
                                     start=(cb == 0), stop=(cb == 7))
                sq = samp2.tile([CM, T2], BF, tag="sq")
                nc.vector.tensor_scalar(kencA[0:CM, :], pk2[:],
                                        2.0 * TEMP, bk2s_s[:],
                                        op0=ALU.mult, op1=ALU.add)
                nc.scalar.activation(sq[:], pk2[:], ACT.Square, bias=bk2_s[:])
                pk2r = ps_m.tile([1, T2], F32, tag="pk2")
                nc.tensor.matmul(pk2r[:], ones80[:], sq[:], start=True, stop=True)
                nc.scalar.activation(kencA[96:97, :], pk2r[:], ACT.Copy,
                                     scale=-TEMP)

                # mask broadcast [T2] -> [128, T2]
                mbc = samp2.tile([128, T2], BF, tag="mbc")
                mrow = msk[b]
                bc = bass.AP(tensor=mrow.tensor, offset=mrow.offset,
                             ap=[[0, 128]] + list(mrow.ap))
                nc.sync.dma_start(mbc[:], bc)

                # ================= queries encoder =================
                xq = samp2.tile([CM, T1], BF, tag="xq")
                nc.sync.dma_start(xq[:], qin[b])
                nc.vector.tensor_scalar_add(xq[:], xq[:], qspk_s[:, b:b + 1])
                h1q = samp1.tile([CM, 2, T1], BF, tag="h1q")
                for ob in range(2):
                    for q in range(4):
                        c0 = q * 512
                        pq = ps_q.tile([CM, 512], F32, tag="pq")
                        first = True
                        for d, lo, hi, ilo in conv3_ranges(T1):
                            glo = max(lo, c0)
                            ghi = min(hi, c0 + 512)
                            gilo = ilo + (glo - lo)
                            nc.tensor.matmul(
                                pq[:, glo - c0:ghi - c0],
                                w1q_s[:, d, ob, :],
                                xq[:, gilo:gilo + (ghi - glo)],
                                start=first, stop=(d == 2))
                            first = False
                        nc.scalar.activation(
                            h1q[:, ob, c0:c0 + 512], pq[:],
                            ACT.Relu, bias=bq1_s[:, ob:ob + 1])
                h2q = samp1.tile([CM, T1], BF, tag="h2q")
                for q in range(4):
                    c0 = q * 512
                    pq = ps_q.tile([CM, 512], F32, tag="pq")
                    for cb in range(2):
                        nc.tensor.matmul(pq[:], w2q_s[:, cb, :],
                                         h1q[:, cb, c0:c0 + 512],
                                         start=(cb == 0), stop=(cb == 1))
                    nc.vector.tensor_scalar(h2q[:, c0:c0 + 512],
                                            pq[:], bq2_s[:], 0.0,
                                            op0=ALU.add, op1=ALU.max)
                for q in range(4):
                    c0 = q * 512
                    pq = ps_q.tile([CM, 512], F32, tag="pq")
                    nc.tensor.matmul(pq[:], w3q_s[:], h2q[:, c0:c0 + 512],
                                     start=True, stop=True)
                    nc.vector.tensor_scalar_add(
                        qencA[0:CM, c0:c0 + 512], pq[:], bq3_s[:])

                return mbc

            def passA(b, mbc):
                qencA = qencA2[b % 2]
                kencA = kencA2[b % 2]
                # ============ logits + double softmax, two passes ============
                ts2a = samp2b.tile([128, NT1, T2], BF, tag="ts2a")
                tma = samp2b.tile([128, NT1, T2], BF, tag="tma")
                s0a = stats.tile([128, NT1], F32, tag="s0a")
                s1a = stats.tile([128, NT1], F32, tag="s1a")
                r0a = stats.tile([128, NT1], F32, tag="r0a")
                r1a = stats.tile([128, NT1], F32, tag="r1a")

                # pass A: matmul -> exp(+rowsum) -> *prior -> *mask(+rowsum)
                for h in range(4):
                    p8 = samp2.tile([128, 4, T2], BF, tag="p8")
                    nc.sync.dma_start(
                        p8[:],
                        prior[b].rearrange("k p s -> p k s")[:, h * 4:h * 4 + 4, :])
                    for j in range(4):
                        t = h * 4 + j
                        pl = ps_l.tile([128, T2], F32, tag="pl")
                        nc.tensor.matmul(pl[:], qencA[:, t * 128:(t + 1) * 128],
                                         kencA[:], start=True, stop=True)
                        el = soft.tile([128, T2], BF, tag="el")
                        nc.scalar.activation(el[:], pl[:], ACT.Exp,
                                             accum_out=s0a[:, t:t + 1])
                        nc.vector.tensor_mul(ts2a[:, t, :], el[:], p8[:, j, :])
                        nc.vector.scalar_tensor_tensor(
                            tma[:, t, :], ts2a[:, t, :], 1.0, mbc[:],
                            op0=ALU.mult, op1=ALU.mult,
                            accum_out=s1a[:, t:t + 1])
                nc.vector.reciprocal(r0a[:], s0a[:])
                nc.vector.reciprocal(r1a[:], s1a[:])
                return ts2a, tma, r0a, r1a

            def passB(b, state):
                ts2a, tma, r0a, r1a = state
                # pass B: lp = ln(ts2/s0); attn = tm/s1
                for hh in range(2):
                    lpa = outp.tile([128, 8, T2], F16, tag="lpa")
                    ata = outp.tile([128, 8, T2], F16, tag="ata")
                    for j in range(8):
                        t = hh * 8 + j
                        nc.scalar.activation(lpa[:, j, :], ts2a[:, t, :], ACT.Ln,
                                             scale=r0a[:, t:t + 1])
                        nc.vector.tensor_scalar_mul(ata[:, j, :], tma[:, t, :],
                                                    r1a[:, t:t + 1])
                    dst = slice(hh * 8, hh * 8 + 8)
                    nc.sync.dma_start(
                        out_lp[b].rearrange("k p s -> p k s")[:, dst, :], lpa[:])
                    nc.sync.dma_start(
                        out_at[b].rearrange("k p s -> p k s")[:, dst, :], ata[:])

            # software-pipelined emission: encoder work of sample b+1 is
            # queued on each engine BEFORE the softmax batch of sample b, so
            # PE-feeding evictions never sit behind a long exp/ln batch.
            mb = {}
            st = {}
            mb[0] = encoders(0)
            mb[1] = encoders(1)
            st[0] = passA(0, mb[0])
            mb[2] = encoders(2)
            st[1] = passA(1, mb[1])
            passB(0, st[0])
            mb[3] = encoders(3)
            st[2] = passA(2, mb[2])
            passB(1, st[1])
            st[3] = passA(3, mb[3])
            passB(2, st[2])
            passB(3, st[3])

    nc.compile()
    return nc


def _get_nc():
    global _nc_cache
    if _nc_cache is None:
        _nc_cache = build_nc()
    return _nc_cache


def prep_inputs(queries, keys, mask, attn_prior, speaker_embed,
                kw1, kb1, kw2, kb2, qw1, qb1, qw2, qb2, qw3, qb3,
                kspk_w, kspk_b, qspk_w, qspk_b):
    """Host-side layout/dtype prep -> list of 8 per-core input maps."""
    f = np.float32
    qh = np.ascontiguousarray(queries, dtype=f).astype(NBF)          # [B,80,2048]
    kh = np.ascontiguousarray(
        np.asarray(keys, dtype=f).reshape(B, 4, 128, T2).transpose(0, 2, 1, 3)
    ).astype(NBF)                                                    # [B,128,4,512]
    ph = (np.asarray(attn_prior, dtype=f) + 1e-8).reshape(
        B, NT1, 128, T2).astype(NBF)                                 # [B,16,128,512]
    mh = (~np.asarray(mask).reshape(B, T2)).astype(f).astype(NBF)    # [B,512]
    sh = np.ascontiguousarray(
        np.asarray(speaker_embed, dtype=f).reshape(B, 4, 128).transpose(2, 1, 0)
    ).astype(NBF)                                                    # [128,4,B]

    pk128 = np.zeros((128, 3008), f)
    pk128[:, 0:2048] = (np.asarray(kspk_w, dtype=f).reshape(4, 128, 4, 128)
                        .transpose(3, 2, 0, 1).reshape(128, 2048))
    pk128[:, 2048:2688] = ((1.0 / (W1K_SCALE * XK_SCALE))
                           * np.asarray(kw2, dtype=f).reshape(CM, 8, 128)
                           .transpose(2, 1, 0).reshape(128, 640))
    pk128[:, 2688:3008] = (np.asarray(qspk_w, dtype=f).reshape(CM, 4, 128)
                           .transpose(2, 1, 0).reshape(128, 320))
    pk80 = np.zeros((CM, 720), f)
    pk80[:, 0:480] = (np.asarray(qw1, dtype=f).reshape(2, CM, CM, 3)
                      .transpose(2, 3, 0, 1).reshape(CM, 480))
    pk80[:, 480:640] = (np.asarray(qw2, dtype=f).reshape(CM, 2, CM)
                        .transpose(2, 1, 0).reshape(CM, 160))
    pk80[:, 640:720] = np.asarray(qw3, dtype=f).reshape(CM, CM).T
    pkf = np.zeros((128, 19), f)
    pkf[:, 0:8] = (W1K_SCALE * XK_SCALE) * np.asarray(kb1, dtype=f).reshape(8, 128).T
    pkf[:, 8:12] = np.asarray(kspk_b, dtype=f).reshape(4, 128).T
    pkf[0:CM, 12] = np.asarray(kb2, dtype=f)
    pkf[0:CM, 13] = 2.0 * TEMP * np.asarray(kb2, dtype=f)
    pkf[0:CM, 14:16] = np.asarray(qb1, dtype=f).reshape(2, CM).T
    pkf[0:CM, 16] = np.asarray(qb2, dtype=f)
    pkf[0:CM, 17] = np.asarray(qb3, dtype=f)
    pkf[0:CM, 18] = np.asarray(qspk_b, dtype=f)
    shared = {
        "w1k": np.ascontiguousarray(
            W1K_SCALE * np.asarray(kw1, dtype=f).reshape(8, 128, 4, 128, 3)
            .transpose(3, 4, 2, 0, 1)).astype(NF8),
        "pk128": pk128.astype(NBF),
        "pk80": pk80.astype(NBF),
        "pkf": pkf,
    }
    in_maps = []
    for c in range(NCORES):
        s = slice(c * BL, (c + 1) * BL)
        m = dict(shared)
        m["qin"] = np.ascontiguousarray(qh[s])
        m["kin"] = np.ascontiguousarray(kh[s])
        m["prior"] = np.ascontiguousarray(ph[s])
        m["msk"] = np.ascontiguousarray(mh[s])
        m["spkT"] = np.ascontiguousarray(sh[:, :, s])
        in_maps.append(m)
    return in_maps


def assemble(results):
    attn = np.empty((B, 1, T1, T2), np.float32)
    lp = np.empty((B, 1, T1, T2), np.float32)
    for c in range(NCORES):
        r = results[c]
        lp[c * BL:(c + 1) * BL, 0] = (
            r["out_lp"].astype(np.float32).reshape(BL, T1, T2))
        attn[c * BL:(c + 1) * BL, 0] = (
            r["out_at"].astype(np.float32).reshape(BL, T1, T2))
    return attn, lp


def kernel(queries, keys, mask, attn_prior, speaker_embed,
           kw1, kb1, kw2, kb2, qw1, qb1, qw2, qb2, qw3, qb3,
           kspk_w, kspk_b, qspk_w, qspk_b, _trace=False):
    nc = _get_nc()
    in_maps = prep_inputs(queries, keys, mask, attn_prior, speaker_embed,
                          kw1, kb1, kw2, kb2, qw1, qb1, qw2, qb2, qw3, qb3,
                          kspk_w, kspk_b, qspk_w, qspk_b)
    res = run_bass_kernel_spmd(nc, in_maps, list(range(NCORES)), trace=_trace)
    attn, lp = assemble(res.results)
    if _trace:
        kernel.last_exec_time_ns = res.exec_time_ns
        kernel.last_result = res
    return attn, lp


# revision 23
# speedup vs baseline: 1.1983x; 1.1983x over previous
# Trainium2 Bass kernel for nn_AlignmentEncoder (RAD-TTS style alignment encoder).
#
# Math (per sample):
#   k_spk = kspk_w @ spk + kspk_b ; q_spk = qspk_w @ spk + qspk_b
#   keys_enc = Conv1x(ReLU(Conv3(keys + k_spk)))                      [80, 512]
#   queries_enc = Conv1x(ReLU(Conv1x(ReLU(Conv3(queries + q_spk)))))  [80, 2048]
#   logits = -T*(q2 + k2 - 2 qk) ; lp = log_softmax(logits) + log(prior + 1e-8)
#   attn = softmax(where(mask, -1e9, lp))
#
# Device-side simplifications (exact up to fp rounding):
#   * q2 (per-row constant) cancels in both log_softmax and softmax -> dropped.
#   * L := 2T*qk - T*k2 computed as ONE matmul with K=97 (k2 folded as an
#     extra contraction row at partition 96 against a ones-row in the query
#     operand; partitions 80..95 are zeroed).
#   * no row-max subtraction: L ranges ~[-1, +1], exp() is safe.
#   * exp(L + log p) == exp(L) * p   ->  no log-prior add per tile:
#       el = exp(L), s0 = sum(el);  ts = el * p;  lp_out = ln(ts / s0)
#       tm = ts * mask01;  attn = tm / sum(tm)   (the 1/s0 factor cancels)
#   * conv k=3 edges via ragged per-tap matmul output ranges (PSUM
#     has_written handles partial-range accumulation) - no zero padding.
#   * Exp and Ln batched into separate passes per sample (ACT function
#     tables for exp and ln live in different table sets; interleaving
#     would reload the table per instruction, ~1.3us each).
#
# Sharding: pure data-parallel, batch 32 = 8 cores x 4 samples. No collectives.
import sys

if "/opt/trn_rl_repo" not in sys.path:
    sys.path.insert(0, "/opt/trn_rl_repo")

import numpy as np
import ml_dtypes

import concourse.bass as bass
import concourse.bacc as bacc
import concourse.tile as tile
from concourse import mybir
from concourse.bass_utils import run_bass_kernel_spmd

BF = mybir.dt.bfloat16
F32 = mybir.dt.float32
F16 = mybir.dt.float16
F8 = mybir.dt.float8e4
NBF = ml_dtypes.bfloat16
NF8 = ml_dtypes.float8_e4m3
W1K_SCALE = 8.0
XK_SCALE = 4.0

TEMP = 0.0005
B, T1, T2 = 32, 2048, 512
CM, CK = 80, 512           # n_mel/n_att, n_text
NCORES, BL = 8, 4          # cores, samples per core
NT1 = T1 // 128            # 16 t1-tiles per sample
ACT = mybir.ActivationFunctionType
ALU = mybir.AluOpType

_nc_cache = None


def conv3_ranges(t_total):
    """Per-tap (d, out_lo, out_hi, in_lo) for a k=3 'same' conv as matmuls.
    Tap d multiplies x[t + d - 1]; ragged output ranges at the edges."""
    out = []
    for d in (1, 0, 2):  # d=1 first: full range, so start=True covers the bank
        lo = max(0, 1 - d)                 # t+d-1 >= 0
        hi = min(t_total, t_total + 1 - d)  # t+d-1 < t_total
        out.append((d, lo, hi, lo + d - 1))
    return out


def build_nc():
    nc = bacc.Bacc("TRN2", target_bir_lowering=False, debug=False,
                   num_devices=NCORES)

    def din(name, shape, dt):
        return nc.dram_tensor(name, list(shape), dt, kind="ExternalInput").ap()

    def dout(name, shape, dt):
        return nc.dram_tensor(name, list(shape), dt, kind="ExternalOutput").ap()

    # -------- external tensors (layouts are partition-first; see host prep)
    qin = din("qin", (BL, CM, T1), BF)              # queries (bf16)
    kin = din("kin", (BL, 128, 4, T2), BF)          # keys [b, p, ci_blk, t]
    prior = din("prior", (BL, NT1, 128, T2), BF)    # attn_prior + 1e-8, tiled
    msk = din("msk", (BL, T2), BF)                  # 1.0 valid / 0.0 masked
    spkT = din("spkT", (128, 4, BL), BF)            # speaker_embed^T blocks
    w1k = din("w1k", (128, 3, 4, 8, 128), F8)       # kw1 [ci_p, d, ci_blk, co_blk, co]
    pk128 = din("pk128", (128, 3008), BF)           # wkspk | w2k | wqspk packed
    pk80 = din("pk80", (CM, 720), BF)               # w1q | w2q | w3q packed
    pkf = din("pkf", (128, 19), F32)                # biases packed
    out_lp = dout("out_lp", (BL, NT1, 128, T2), F16)
    out_at = dout("out_at", (BL, NT1, 128, T2), F16)

    with tile.TileContext(nc) as tc:
        import contextlib
        with contextlib.ExitStack() as ctx:
            wts = ctx.enter_context(tc.tile_pool(name="wts", bufs=1))
            samp2 = ctx.enter_context(tc.tile_pool(name="samp2", bufs=2))
            samp1 = ctx.enter_context(tc.tile_pool(name="samp1", bufs=1))
            samp2b = ctx.enter_context(tc.tile_pool(name="samp2b", bufs=2))
            outp = ctx.enter_context(tc.tile_pool(name="outp", bufs=2))
            soft = ctx.enter_context(tc.tile_pool(name="soft", bufs=4))
            stats = ctx.enter_context(tc.tile_pool(name="stats", bufs=2))
            ps_l = ctx.enter_context(tc.tile_pool(name="ps_l", bufs=3, space="PSUM"))
            ps_k1 = ctx.enter_context(tc.tile_pool(name="ps_k1", bufs=2, space="PSUM"))
            ps_m = ctx.enter_context(tc.tile_pool(name="ps_m", bufs=1, space="PSUM"))
            ps_q = ctx.enter_context(tc.tile_pool(name="ps_q", bufs=2, space="PSUM"))

            # -------- static weights into SBUF
            def wtile(ap_in, shape, dt, tag):
                t = wts.tile(list(shape), dt, tag=tag)
                nc.sync.dma_start(t[:], ap_in[:])
                return t

            spkT_s = wtile(spkT, (128, 4, BL), BF, "spkT")
            pkf_s = wtile(pkf, (128, 19), F32, "pkf")
            pk128_s = wtile(pk128, (128, 3008), BF, "pk128")
            pk80_s = wtile(pk80, (CM, 720), BF, "pk80")
            w1k_s = wtile(w1k, (128, 3, 4, 8, 128), F8, "w1k")
            wkspk_s = pk128_s[:, 0:2048].rearrange("p (a b c) -> p a b c",
                                                   a=4, b=4)
            w2k_s = pk128_s[:, 2048:2688].rearrange("p (a b) -> p a b", a=8)
            wqspk_s = pk128_s[:, 2688:3008].rearrange("p (a b) -> p a b", a=4)
            w1q_s = pk80_s[:, 0:480].rearrange("p (a b c) -> p a b c", a=3, b=2)
            w2q_s = pk80_s[:, 480:640].rearrange("p (a b) -> p a b", a=2)
            w3q_s = pk80_s[:, 640:720]
            bk1_s = pkf_s[:, 0:8]
            bkspk_s = pkf_s[:, 8:12]
            bk2_s = pkf_s[0:CM, 12:13]
            bk2s_s = pkf_s[0:CM, 13:14]
            bq1_s = pkf_s[0:CM, 14:16]
            bq2_s = pkf_s[0:CM, 16:17]
            bq3_s = pkf_s[0:CM, 17:18]
            bqspk_s = pkf_s[0:CM, 18:19]

            ones80 = wts.tile([CM, 1], BF, tag="ones80")
            nc.gpsimd.memset(ones80[:], 1.0)

            # static double-buffered augmented encoder outputs; filler rows
            # (80..95 zero) + ones row (96) written once.
            qencA2, kencA2 = [], []
            for i in range(2):
                qe = wts.tile([97, T1], BF, tag=f"qencA{i}")
                nc.gpsimd.memset(qe[64:96, :], 0.0)
                nc.gpsimd.memset(qe[96:97, :], 1.0)
                qencA2.append(qe)
                ke = wts.tile([97, T2], BF, tag=f"kencA{i}")
                nc.gpsimd.memset(ke[64:96, :], 0.0)
                kencA2.append(ke)

            # -------- speaker projections (all local samples at once)
            kspk_s = wts.tile([128, 4, BL], F32, tag="kspk")   # [c_p, c_blk, b]
            qspk_s = wts.tile([CM, BL], F32, tag="qspk")
            for cb in range(4):
                ps = ps_k1.tile([128, BL], F32, tag="pk1")
                for jb in range(4):
                    nc.tensor.matmul(ps[:], wkspk_s[:, jb, cb, :], spkT_s[:, jb, :],
                                     start=(jb == 0), stop=(jb == 3))
                nc.scalar.activation(kspk_s[:, cb, :], ps[:], ACT.Identity,
                                     bias=bkspk_s[:, cb:cb + 1])
            psq = ps_m.tile([CM, BL], F32, tag="pk2")
            for jb in range(4):
                nc.tensor.matmul(psq[:], wqspk_s[:, jb, :], spkT_s[:, jb, :],
                                 start=(jb == 0), stop=(jb == 3))
            nc.scalar.activation(qspk_s[:], psq[:], ACT.Identity, bias=bqspk_s[:])

            def encoders(b):
                qencA = qencA2[b % 2]
                kencA = kencA2[b % 2]
                # ================= keys encoder =================
                xk = samp2.tile([128, 4, T2], BF, tag="xk")
                nc.sync.dma_start(xk[:], kin[b])
                xk8 = samp2.tile([128, 4, T2], F8, tag="xk8")
                for cb in range(4):
                    # xk8 = (keys + kspk) * XK_SCALE, in fp8 for DoubleRow
                    nc.vector.tensor_scalar(xk8[:, cb, :], xk[:, cb, :],
                                            kspk_s[:, cb, b:b + 1], XK_SCALE,
                                            op0=ALU.add, op1=ALU.mult)
                h1k = samp1.tile([128, 8, T2], BF, tag="h1k")
                for ob in range(8):
                    pk = ps_k1.tile([128, T2], F32, tag="pk1")
                    first = True
                    for cbp in range(2):
                        for d, lo, hi, ilo in conv3_ranges(T2):
                            nc.tensor.matmul(
                                pk[:, lo:hi],
                                w1k_s[:, d, 2 * cbp:2 * cbp + 2, ob, :],
                                xk8[:, 2 * cbp:2 * cbp + 2, ilo:ilo + (hi - lo)],
                                start=first, stop=(cbp == 1 and d == 2),
                                perf_mode=mybir.MatmulPerfMode.DoubleRow)
                            first = False
                    nc.vector.tensor_scalar(h1k[:, ob, :], pk[:],
                                            bk1_s[:, ob:ob + 1], 0.0,
                                            op0=ALU.add, op1=ALU.max)
                pk2 = ps_m.tile([CM, T2], F32, tag="pk2")
                for cb in range(8):
                    nc.tensor.matmul(pk2[:], w2k_s[:, cb, :], h1k[:, cb, :],
                                     start=(cb == 0), stop=(cb == 7))
                sq = samp2.tile([CM, T2], BF, tag="sq")
                nc.vector.tensor_scalar(kencA[0:CM, :], pk2[:],
                                        2.0 * TEMP, bk2s_s[:],
                                        op0=ALU.mult, op1=ALU.add)
                nc.scalar.activation(sq[:], pk2[:], ACT.Square, bias=bk2_s[:])
                pk2r = ps_m.tile([1, T2], F32, tag="pk2")
                nc.tensor.matmul(pk2r[:], ones80[:], sq[:], start=True, stop=True)
                nc.scalar.activation(kencA[96:97, :], pk2r[:], ACT.Copy,
                                     scale=-TEMP)

                # mask broadcast [T2] -> [128, T2]
                mbc = samp2.tile([128, T2], BF, tag="mbc")
                mrow = msk[b]
                bc = bass.AP(tensor=mrow.tensor, offset=mrow.offset,
                             ap=[[0, 128]] + list(mrow.ap))
                nc.gpsimd.dma_start(mbc[:], bc)

                # ================= queries encoder =================
                xq = samp2.tile([CM, T1], BF, tag="xq")
                nc.sync.dma_start(xq[:], qin[b])
                nc.vector.tensor_scalar_add(xq[:], xq[:], qspk_s[:, b:b + 1])
                h1q = samp1.tile([CM, 2, T1], BF, tag="h1q")
                for ob in range(2):
                    for q in range(4):
                        c0 = q * 512
                        pq = ps_q.tile([CM, 512], F32, tag="pq")
                        first = True
                        for d, lo, hi, ilo in conv3_ranges(T1):
                            glo = max(lo, c0)
                            ghi = min(hi, c0 + 512)
                            gilo = ilo + (glo - lo)
                            nc.tensor.matmul(
                                pq[:, glo - c0:ghi - c0],
                                w1q_s[:, d, ob, :],
                                xq[:, gilo:gilo + (ghi - glo)],
                                start=first, stop=(d == 2))
                            first = False
                        nc.scalar.activation(
                            h1q[:, ob, c0:c0 + 512], pq[:],
                            ACT.Relu, bias=bq1_s[:, ob:ob + 1])
                h2q = samp1.tile([CM, T1], BF, tag="h2q")
                for q in range(4):
                    c0 = q * 512
                    pq = ps_q.tile([CM, 512], F32, tag="pq")
                    for cb in range(2):
                        nc.tensor.matmul(pq[:], w2q_s[:, cb, :],
                                         h1q[:, cb, c0:c0 + 512],
                                         start=(cb == 0), stop=(cb == 1))
                    nc.vector.tensor_scalar(h2q[:, c0:c0 + 512],
                                            pq[:], bq2_s[:], 0.0,
                                            op0=ALU.add, op1=ALU.max)
                for q in range(4):
                    c0 = q * 512
                    pq = ps_q.tile([CM, 512], F32, tag="pq")
                    nc.tensor.matmul(pq[:], w3q_s[:], h2q[:, c0:c0 + 512],
                                     start=True, stop=True)
                    nc.vector.tensor_scalar_add(
                        qencA[0:CM, c0:c0 + 512], pq[:], bq3_s[:])

                return mbc

            def passA(b, mbc):
                qencA = qencA2[b % 2]
                kencA = kencA2[b % 2]
                # ============ logits + double softmax, two passes ============
                ts2a = samp2b.tile([128, NT1, T2], BF, tag="ts2a")
                tma = samp2b.tile([128, NT1, T2], BF, tag="tma")
                s0a = stats.tile([128, NT1], F32, tag="s0a")
                s1a = stats.tile([128, NT1], F32, tag="s1a")
                r0a = stats.tile([128, NT1], F32, tag="r0a")
                r1a = stats.tile([128, NT1], F32, tag="r1a")

                # pass A: matmul -> exp(+rowsum) -> *prior -> *mask(+rowsum)
                for h in range(4):
                    p8 = samp2.tile([128, 4, T2], BF, tag="p8")
                    nc.sync.dma_start(
                        p8[:],
                        prior[b].rearrange("k p s -> p k s")[:, h * 4:h * 4 + 4, :])
                    for j in range(4):
                        t = h * 4 + j
                        pl = ps_l.tile([128, T2], F32, tag="pl")
                        nc.tensor.matmul(pl[:], qencA[:, t * 128:(t + 1) * 128],
                                         kencA[:], start=True, stop=True)
                        el = soft.tile([128, T2], BF, tag="el")
                        nc.scalar.activation(el[:], pl[:], ACT.Exp,
                                             accum_out=s0a[:, t:t + 1])
                        nc.vector.tensor_mul(ts2a[:, t, :], el[:], p8[:, j, :])
                        nc.vector.scalar_tensor_tensor(
                            tma[:, t, :], ts2a[:, t, :], 1.0, mbc[:],
                            op0=ALU.mult, op1=ALU.mult,
                            accum_out=s1a[:, t:t + 1])
                nc.vector.reciprocal(r0a[:], s0a[:])
                nc.vector.reciprocal(r1a[:], s1a[:])
                return ts2a, tma, r0a, r1a

            def passB(b, state):
                ts2a, tma, r0a, r1a = state
                # pass B: lp = ln(ts2/s0); attn = tm/s1
                for hh in range(2):
                    lpa = outp.tile([128, 8, T2], F16, tag="lpa")
                    ata = outp.tile([128, 8, T2], F16, tag="ata")
                    for j in range(8):
                        t = hh * 8 + j
                        nc.scalar.activation(lpa[:, j, :], ts2a[:, t, :], ACT.Ln,
                                             scale=r0a[:, t:t + 1])
                        nc.vector.tensor_scalar_mul(ata[:, j, :], tma[:, t, :],
                                                    r1a[:, t:t + 1])
                    dst = slice(hh * 8, hh * 8 + 8)
                    nc.sync.dma_start(
                        out_lp[b].rearrange("k p s -> p k s")[:, dst, :], lpa[:])
                    nc.sync.dma_start(
                        out_at[b].rearrange("k p s -> p k s")[:, dst, :], ata[:])

            # software-pipelined emission: encoder work of sample b+1 is
            # queued on each engine BEFORE the softmax batch of sample b, so
            # PE-feeding evictions never sit behind a long exp/ln batch.
            mb = {}
            st = {}
            mb[0] = encoders(0)
            mb[1] = encoders(1)
            st[0] = passA(0, mb[0])
            mb[2] = encoders(2)
            st[1] = passA(1, mb[1])
            passB(0, st[0])
            mb[3] = encoders(3)
            st[2] = passA(2, mb[2])
            passB(1, st[1])
            st[3] = passA(3, mb[3])
            passB(2, st[2])
            passB(3, st[3])

    nc.compile()
    return nc


def _get_nc():
    global _nc_cache
    if _nc_cache is None:
        _nc_cache = build_nc()
    return _nc_cache


def prep_inputs(queries, keys, mask, attn_prior, speaker_embed,
                kw1, kb1, kw2, kb2, qw1, qb1, qw2, qb2, qw3, qb3,
                kspk_w, kspk_b, qspk_w, qspk_b):
    """Host-side layout/dtype prep -> list of 8 per-core input maps."""
    f = np.float32
    qh = np.ascontiguousarray(queries, dtype=f).astype(NBF)          # [B,80,2048]
    kh = np.ascontiguousarray(
        np.asarray(keys, dtype=f).reshape(B, 4, 128, T2).transpose(0, 2, 1, 3)
    ).astype(NBF)                                                    # [B,128,4,512]
    ph = (np.asarray(attn_prior, dtype=f) + 1e-8).reshape(
        B, NT1, 128, T2).astype(NBF)                                 # [B,16,128,512]
    mh = (~np.asarray(mask).reshape(B, T2)).astype(f).astype(NBF)    # [B,512]
    sh = np.ascontiguousarray(
        np.asarray(speaker_embed, dtype=f).reshape(B, 4, 128).transpose(2, 1, 0)
    ).astype(NBF)                                                    # [128,4,B]

    pk128 = np.zeros((128, 3008), f)
    pk128[:, 0:2048] = (np.asarray(kspk_w, dtype=f).reshape(4, 128, 4, 128)
                        .transpose(3, 2, 0, 1).reshape(128, 2048))
    pk128[:, 2048:2688] = ((1.0 / (W1K_SCALE * XK_SCALE))
                           * np.asarray(kw2, dtype=f).reshape(CM, 8, 128)
                           .transpose(2, 1, 0).reshape(128, 640))
    pk128[:, 2688:3008] = (np.asarray(qspk_w, dtype=f).reshape(CM, 4, 128)
                           .transpose(2, 1, 0).reshape(128, 320))
    pk80 = np.zeros((CM, 720), f)
    pk80[:, 0:480] = (np.asarray(qw1, dtype=f).reshape(2, CM, CM, 3)
                      .transpose(2, 3, 0, 1).reshape(CM, 480))
    pk80[:, 480:640] = (np.asarray(qw2, dtype=f).reshape(CM, 2, CM)
                        .transpose(2, 1, 0).reshape(CM, 160))
    pk80[:, 640:720] = np.asarray(qw3, dtype=f).reshape(CM, CM).T
    pkf = np.zeros((128, 19), f)
    pkf[:, 0:8] = (W1K_SCALE * XK_SCALE) * np.asarray(kb1, dtype=f).reshape(8, 128).T
    pkf[:, 8:12] = np.asarray(kspk_b, dtype=f).reshape(4, 128).T
    pkf[0:CM, 12] = np.asarray(kb2, dtype=f)
    pkf[0:CM, 13] = 2.0 * TEMP * np.asarray(kb2, dtype=f)
    pkf[0:CM, 14:16] = np.asarray(qb1, dtype=f).reshape(2, CM).T
    pkf[0:CM, 16] = np.asarray(qb2, dtype=f)
    pkf[0:CM, 17] = np.asarray(qb3, dtype=f)
    pkf[0:CM, 18] = np.asarray(qspk_b, dtype=f)
    shared = {
        "w1k": np.ascontiguousarray(
            W1K_SCALE * np.asarray(kw1, dtype=f).reshape(8, 128, 4, 128, 3)
            .transpose(3, 4, 2, 0, 1)).astype(NF8),
        "pk128": pk128.astype(NBF),
        "pk80": pk80.astype(NBF),
        "pkf": pkf,
    }
    in_maps = []
    for c in range(NCORES):
        s = slice(c * BL, (c + 1) * BL)
        m = dict(shared)
        m["qin"] = np.ascontiguousarray(qh[s])
        m["kin"] = np.ascontiguousarray(kh[s])
        m["prior"] = np.ascontiguousarray(ph[s])
        m["msk"] = np.ascontiguousarray(mh[s])
        m["spkT"] = np.ascontiguousarray(sh[:, :, s])
        in_maps.append(m)
    return in_maps


def assemble(results):
    attn = np.empty((B, 1, T1, T2), np.float32)
    lp = np.empty((B, 1, T1, T2), np.float32)
    for c in range(NCORES):
        r = results[c]
        lp[c * BL:(c + 1) * BL, 0] = (
            r["out_lp"].astype(np.float32).reshape(BL, T1, T2))
        attn[c * BL:(c + 1) * BL, 0] = (
            r["out_at"].astype(np.float32).reshape(BL, T1, T2))
    return attn, lp


def kernel(queries, keys, mask, attn_prior, speaker_embed,
           kw1, kb1, kw2, kb2, qw1, qb1, qw2, qb2, qw3, qb3,
           kspk_w, kspk_b, qspk_w, qspk_b, _trace=False):
    nc = _get_nc()
    in_maps = prep_inputs(queries, keys, mask, attn_prior, speaker_embed,
                          kw1, kb1, kw2, kb2, qw1, qb1, qw2, qb2, qw3, qb3,
                          kspk_w, kspk_b, qspk_w, qspk_b)
    res = run_bass_kernel_spmd(nc, in_maps, list(range(NCORES)), trace=_trace)
    attn, lp = assemble(res.results)
    if _trace:
        kernel.last_exec_time_ns = res.exec_time_ns
        kernel.last_result = res
    return attn, lp
